# revision 7
# baseline (speedup 1.0000x reference)
"""AutoCorrelation kernel for Trainium2 (8 NeuronCores, SPMD data-parallel over batch).

Math (derived from the reference nn.Module):
  - R = irfft(rfft(Q) * conj(rfft(K))) is a circular cross-correlation; the
    reference reduces it with mean over (heads, ALL lags).  Sum over all lags
    of a circular cross-correlation factorizes:  sum_tau R[tau] =
    (sum_t Q[t]) * (sum_s K[s]).  So the FFT is algebraically unnecessary --
    only column sums of Q and K are needed, and those are linear in the
    column sums of q and k (sum_t(q @ Wq + bq) = (sum_t q) @ Wq + L*bq).
  - The top-k "delays" are channel indices in [0, 64).  The delay aggregation
    sum_i w_i * roll(V, -d_i) commutes with the output projection AND with the
    value projection, so:  out[t] = sum_d coef_d * U[(t+d) % L]  where
    U = v @ (Wv @ Wo), plus bias (bv @ Wo + bo).  The tap sum is a 64-band
    Toeplitz matmul on the tensor engine.
  - The band coefficients sum to exactly 1 (softmax), so the bias can be
    folded into U (out = bands^T (U + bias) = bands^T U + bias) -- the bias
    add rides the PSUM->SBUF cast instead of the conv output path.

Device work:
  phase 1: column sums of q[b], k[b] per core via ones-vector matmuls
           (memory bound; bf16 inputs, fp32 PSUM accumulation)
  phase 2: U = v @ W2 + bias per 128-row tile, then out_i = band1^T U_i +
           band2^T U_{i+1} (circular), stored as bf16 (host upcasts).
           PE warm-up matmuls at kernel start remove the HAM cold-clock
           penalty while the first vT slices are still in flight.
Host work: [8,512]@[512,512] glue matmuls, top-41 of 64, softmax, band build.
"""

import sys

sys.path.insert(0, "/opt/trn_rl_repo")

import numpy as np

import concourse.bass as bass
import concourse.bacc as bacc
import concourse.mybir as mybir
import concourse.tile as tile
from concourse.bass_utils import run_bass_kernel_spmd

B, L, D, H = 8, 4096, 512, 8
DK = D // H          # 64
K_TOP = 41           # min(int(5*log(4096)), 64)
NCORES = 8
F32 = mybir.dt.float32
BF16 = mybir.dt.bfloat16
NP_BF16 = mybir.dt.np(BF16)

# set by test.py to collect HW profiles
PROFILE = False
TRACE_DIR = None
LAST_HW_TIME_NS = {"phase1": None, "phase2": None}

_NC_CACHE = {}


def _make_nc():
    return bacc.Bacc(
        "TRN2", target_bir_lowering=False, debug=False, num_devices=NCORES
    )


def _build_phase1():
    """Per-core: sums[0, :512] = sum_t q[t, :], sums[0, 512:] = sum_t k[t, :].

    q/k arrive as bf16; sums accumulate in fp32 PSUM via ones-vector matmuls.
    DMA plan per stream: 3 x 1 MB + 4 x 0.25 MB tiles -- big early tiles for
    bandwidth, small late tiles so the post-last-byte matmul tail is short.
    """
    nc = _make_nc()
    q = nc.dram_tensor("q", [L, D], BF16, kind="ExternalInput")
    k = nc.dram_tensor("k", [L, D], BF16, kind="ExternalInput")
    sums = nc.dram_tensor("sums", [1, 2 * D], F32, kind="ExternalOutput")

    NBIGSUB = 4              # 0.5 MB tiles
    NSMSUB = 2               # 0.25 MB tail tiles
    NBIG = 7                 # rows 0..3584
    NSM = 2                  # rows 3584..4096
    CHAIN = 2                # DMA chunk g waits on chunk g-CHAIN (ordered arrival)

    with tile.TileContext(nc) as tc:
        with (
            tc.tile_pool(name="singles", bufs=1) as singles,
            tc.tile_pool(name="qk", bufs=NBIG) as qk_pool,
            tc.tile_pool(name="qks", bufs=NSM) as qks_pool,
            tc.tile_pool(name="ps", bufs=2, space=bass.MemorySpace.PSUM) as ps_pool,
        ):
            ones = singles.tile([128, 1], BF16)
            nc.any.memset(ones[:], 1.0)

            q_big = q.ap().rearrange("(g p n) d -> g p n d", p=128, n=NBIGSUB)
            k_big = k.ap().rearrange("(g p n) d -> g p n d", p=128, n=NBIGSUB)
            q_sm = q.ap().rearrange("(g p n) d -> g p n d", p=128, n=NSMSUB)
            k_sm = k.ap().rearrange("(g p n) d -> g p n d", p=128, n=NSMSUB)
            SM0 = NBIG * 2           # first small-group index (rows 3584+)

            chunks = []              # (q_tile, k_tile, nsub)
            qd, kd = [], []          # dma instructions for chaining
            for g in range(NBIG):
                tq = qk_pool.tile([128, NBIGSUB, D], BF16, tag="ldq", name=f"tq{g}")
                qd.append(nc.sync.dma_start(tq[:], q_big[g]))
                tk = qk_pool.tile([128, NBIGSUB, D], BF16, tag="ldk", name=f"tk{g}")
                kd.append(nc.scalar.dma_start(tk[:], k_big[g]))
                chunks.append((tq, tk, NBIGSUB))
            for s in range(NSM):
                tq = qks_pool.tile([128, NSMSUB, D], BF16, tag="sdq", name=f"sq{s}")
                qd.append(nc.sync.dma_start(tq[:], q_sm[SM0 + s]))
                tk = qks_pool.tile([128, NSMSUB, D], BF16, tag="sdk", name=f"sk{s}")
                kd.append(nc.scalar.dma_start(tk[:], k_sm[SM0 + s]))
                chunks.append((tq, tk, NSMSUB))
            for stream in (qd, kd):
                for g in range(CHAIN, len(stream)):
                    tile.add_dep_helper(
                        stream[g].ins,
                        stream[g - CHAIN].ins,
                        sync=True,
                        reason="ordered chunk arrival",
                    )

            psq = ps_pool.tile([1, D], F32)
            psk = ps_pool.tile([1, D], F32)
            ssb = singles.tile([1, 2 * D], F32)
            nchunks = len(chunks)
            for ci, (tq, tk, nsub) in enumerate(chunks):
                first = ci == 0
                last = ci == nchunks - 1
                for c in range(nsub):
                    nc.tensor.matmul(
                        psq[:1, :],
                        ones[:],
                        tq[:, c, :],
                        start=(first and c == 0),
                        stop=(last and c == nsub - 1),
                    )
                for c in range(nsub):
                    nc.tensor.matmul(
                        psk[:1, :],
                        ones[:],
                        tk[:, c, :],
                        start=(first and c == 0),
                        stop=(last and c == nsub - 1),
                    )
                if last:
                    # q's accumulation finished just above; drain and store it
                    # while the final k matmuls run
                    nc.vector.tensor_copy(ssb[:1, 0:D], psq[:1, :])
                    nc.sync.dma_start(sums.ap()[0:1, 0:D], ssb[:1, 0:D])
            nc.vector.tensor_copy(ssb[:1, D : 2 * D], psk[:1, :])
            nc.sync.dma_start(sums.ap()[0:1, D : 2 * D], ssb[:1, D : 2 * D])

    nc.compile()
    return nc


def _build_phase2():
    """Per-core: out[128i + t, n] = sum_s band1[s, t] * U_i[s, n]
                                  + sum_s band2[s, t] * U_{i+1 mod 32}[s, n]
    with U_i = v[128i : 128(i+1), :] @ W2 + bias, from host-transposed vT.
    Output stored bf16 (host upcasts); bias folded into U (bands sum to 1).
    """
    nc = _make_nc()
    vT = nc.dram_tensor("vT", [D, L], BF16, kind="ExternalInput")
    bandsd = nc.dram_tensor("bands", [2, 128, 128], BF16, kind="ExternalInput")
    # host-swizzled halves: w2a[p, n] = W2[p, n]; w2b[p, c*D + n] = W2[(c+1)*128 + p, n]
    w2ad = nc.dram_tensor("w2a", [128, D], BF16, kind="ExternalInput")
    w2bd = nc.dram_tensor("w2b", [128, 3 * D], BF16, kind="ExternalInput")
    biasd = nc.dram_tensor("bias", [128, D], F32, kind="ExternalInput")
    out = nc.dram_tensor("out", [L, D], BF16, kind="ExternalOutput")
    warm = nc.dram_tensor("warm", [1, 4], F32, kind="ExternalOutput")

    NBLK = L // 128          # 32 tiles / output blocks
    OSUB = 2                 # output blocks per store DMA
    # vT arrives per channel group in geometric column levels: tiny early
    # levels unblock the matmul stream at ~9.5us; the bulk is dep-chained
    # behind them so concurrent-DMA fair-sharing cannot starve the stream.
    LEVELS = [256, 256, 512, 1024, 2048]
    PRE_U = 8                # U tiles emitted before the first conv block
    NWARM = 20               # PE warm-up matmuls (HAM ramp) before real work

    with tile.TileContext(nc) as tc:
        with (
            tc.tile_pool(name="singles", bufs=1) as singles,
            tc.tile_pool(name="usb", bufs=PRE_U + 3) as u_pool,
            tc.tile_pool(name="op", bufs=3) as opool,
            tc.tile_pool(name="wps", bufs=1, space=bass.MemorySpace.PSUM) as wps_pool,
            tc.tile_pool(name="ups", bufs=4, space=bass.MemorySpace.PSUM) as ups_pool,
            tc.tile_pool(name="ops", bufs=3, space=bass.MemorySpace.PSUM) as ops_pool,
        ):
            # ---- PE warm-up: no data deps, fills the HAM ramp while the
            # first vT slices are in flight.  A tiny store keeps it live.
            # Small N so the queue drains just as the first real operands
            # land (~2us of PE-busy). ----
            wtile = singles.tile([128, 512], BF16)
            nc.any.memset(wtile[:], 0.0)
            wps = wps_pool.tile([128, 512], F32)
            for j in range(NWARM):
                nc.tensor.matmul(
                    wps[:, 0:128],
                    wtile[:, 0:128],
                    wtile[:, 0:128],
                    start=(j == 0),
                    stop=(j == NWARM - 1),
                )
            wsb = singles.tile([1, 4], F32)
            nc.vector.tensor_copy(wsb[:], wps[0:1, 0:4])
            nc.sync.dma_start(warm.ap(), wsb[:])

            # ---- DMA schedule.  Ring FIFOs matter: concurrent transfers on
            # a ring fair-share bandwidth, so the operands of the first
            # matmuls are first and small on BOTH rings, and later levels are
            # dep-chained behind earlier ones. ----
            vt_re = vT.ap().rearrange("(c p) t -> c p t", p=128)
            w2a_sb = singles.tile([128, D], BF16)
            nc.sync.dma_start(w2a_sb[:], w2ad.ap())
            w2b_sb = singles.tile([128, 3, D], BF16)
            nc.scalar.dma_start(
                w2b_sb[:], w2bd.ap().rearrange("p (c n) -> p c n", c=3)
            )
            vlv = {}                 # (level, cg) -> tile
            vdma = {}                # (level, cg) -> dma inst
            col0 = {}                # level -> first col
            col = 0
            for lv, width in enumerate(LEVELS):
                col0[lv] = col
                for cg in range(4):
                    t = singles.tile([128, width], BF16, name=f"vl{lv}_{cg}")
                    ring = nc.sync if cg % 2 == 0 else nc.scalar
                    vdma[(lv, cg)] = ring.dma_start(
                        t[:], vt_re[cg][:, col : col + width]
                    )
                    vlv[(lv, cg)] = t
                col += width
            bias_sb = singles.tile([128, D], F32)
            bias_dma = nc.scalar.dma_start(bias_sb[:], biasd.ap())
            band_sb = singles.tile([128, 2, 128], BF16)
            band_dma = nc.scalar.dma_start(
                band_sb[:], bandsd.ap().rearrange("b p t -> p b t")
            )
            # chain: level lv waits on level lv-2 (same cg); bias/bands after L1
            for lv in range(2, len(LEVELS)):
                for cg in range(4):
                    tile.add_dep_helper(
                        vdma[(lv, cg)].ins,
                        vdma[(lv - 2, cg)].ins,
                        sync=True,
                        reason="ordered level arrival",
                    )
            tile.add_dep_helper(
                bias_dma.ins, vdma[(1, 1)].ins, sync=True, reason="bias after L1"
            )
            tile.add_dep_helper(
                band_dma.ins, vdma[(1, 3)].ins, sync=True, reason="bands after L1"
            )

            out_re = out.ap().rearrange("(g n p) d -> g p n d", p=128, n=OSUB)

            lvl_of = []              # tile index -> level
            for lv, width in enumerate(LEVELS):
                lvl_of.extend([lv] * (width // 128))

            def u_src(i, cg):
                lv = lvl_of[i]
                off = i * 128 - col0[lv]
                t = vlv[(lv, cg)]
                return t[:, off : off + 128]

            def u_mm_w2(cg):
                return w2a_sb[:] if cg == 0 else w2b_sb[:, cg - 1, :]

            def u_tile(i):
                ups = ups_pool.tile([128, D], F32, tag="ups", name=f"ups{i}")
                for cg in range(4):
                    nc.tensor.matmul(
                        ups[:],
                        u_src(i, cg),
                        u_mm_w2(cg),
                        start=(cg == 0),
                        stop=(cg == 3),
                    )
                # PSUM -> SBUF with the bias folded in and a bf16 downcast
                usb = u_pool.tile([128, D], BF16, tag="usb", name=f"usb{i}")
                nc.vector.tensor_add(usb[:], ups[:], bias_sb[:])
                return usb

            U = {}
            for i in range(PRE_U):
                U[i] = u_tile(i)
            u_first = singles.tile([128, D], BF16)
            nc.vector.tensor_copy(u_first[:], U[0][:])

            ot_tiles = {}
            for i in range(NBLK):
                g, n4 = divmod(i, OSUB)
                if g not in ot_tiles:
                    ot_tiles[g] = opool.tile(
                        [128, OSUB, D], BF16, tag="out", name=f"ot{g}"
                    )
                if i + PRE_U < NBLK:
                    U[i + PRE_U] = u_tile(i + PRE_U)
                u_n = U[i + 1] if i < NBLK - 1 else u_first
                ops = ops_pool.tile([128, D], F32, tag="ops", name=f"ops{i}")
                nc.tensor.matmul(
                    ops[:], band_sb[:, 0, :], U[i][:], start=True, stop=False
                )
                nc.tensor.matmul(
                    ops[:], band_sb[:, 1, :], u_n[:], start=False, stop=True
                )
                del U[i]
                ot = ot_tiles[g]
                nc.scalar.copy(ot[:, n4, :], ops[:])  # ACT: fp32 PSUM -> bf16 SBUF
                if n4 == OSUB - 1:
                    nc.sync.dma_start(out_re[g], ot[:])
                    del ot_tiles[g]

    nc.compile()
    return nc


_RUN_COUNTER = [0]


def _run(nc, in_maps, phase):
    kwargs = {}
    if PROFILE:
        kwargs["trace"] = True
        if TRACE_DIR is not None:
            import os

            _RUN_COUNTER[0] += 1
            d = os.path.join(TRACE_DIR, f"{phase}_{_RUN_COUNTER[0]}")
            os.makedirs(d, exist_ok=True)
            kwargs["tmpdir"] = d
    res = run_bass_kernel_spmd(nc, in_maps, core_ids=list(range(NCORES)), **kwargs)
    LAST_HW_TIME_NS[phase] = res.exec_time_ns
    return res.results


def kernel(q, k, v, Wq, bq, Wk, bk, Wv, bv, Wo, bo):
    q = np.asarray(q, dtype=np.float32)
    k = np.asarray(k, dtype=np.float32)
    v = np.asarray(v, dtype=np.float32)
    Wq, bq, Wk, bk, Wv, bv, Wo, bo = (
        np.asarray(x, dtype=np.float64) for x in (Wq, bq, Wk, bk, Wv, bv, Wo, bo)
    )

    # ---- phase 1: per-batch column sums of q and k (device) ----
    if "p1" not in _NC_CACHE:
        _NC_CACHE["p1"] = _build_phase1()
    q_bf = q.astype(NP_BF16)
    k_bf = k.astype(NP_BF16)
    in_maps = [{"q": q_bf[b], "k": k_bf[b]} for b in range(B)]
    res1 = _run(_NC_CACHE["p1"], in_maps, "phase1")
    sq = np.stack([res1[b]["sums"][0, :D] for b in range(B)]).astype(np.float64)
    sk = np.stack([res1[b]["sums"][0, D:] for b in range(B)]).astype(np.float64)

    # ---- host glue: top-k channel selection + softmax weights ----
    SQ = sq @ Wq + L * bq                       # [B, D]
    SK = sk @ Wk + L * bk
    m = (SQ.reshape(B, H, DK) * SK.reshape(B, H, DK)).sum(axis=1) / (H * L)  # [B, DK]
    mbar = m.mean(axis=0)
    idx = np.argsort(-mbar, kind="stable")[:K_TOP]
    msel = m[:, idx]
    e = np.exp(msel - msel.max(axis=1, keepdims=True))
    w = e / e.sum(axis=1, keepdims=True)        # [B, K_TOP]
    coef = np.zeros((B, DK))
    coef[:, idx] = w

    # Toeplitz bands: out[t] = sum_d coef[d] * U[(t + d) % L]
    s = np.arange(128)[:, None]
    t = np.arange(128)[None, :]
    d1 = s - t
    d2 = s + 128 - t
    m1 = (d1 >= 0) & (d1 < DK)
    m2 = (d2 >= 0) & (d2 < DK)
    bands = np.zeros((B, 2, 128, 128), dtype=np.float64)
    for b in range(B):
        bands[b, 0] = np.where(m1, coef[b][np.clip(d1, 0, DK - 1)], 0.0)
        bands[b, 1] = np.where(m2, coef[b][np.clip(d2, 0, DK - 1)], 0.0)

    W2 = (Wv @ Wo).astype(np.float32)
    bias2 = (bv @ Wo + bo).astype(np.float32)
    bias_rep = np.ascontiguousarray(np.broadcast_to(bias2, (128, D)))
    # split + swizzle: w2a = rows 0..128; w2b rows 128..512 contiguous per chunk
    w2a_bf = np.ascontiguousarray(W2[0:128]).astype(NP_BF16)
    w2b_bf = np.ascontiguousarray(
        W2[128:].reshape(3, 128, D).transpose(1, 0, 2).reshape(128, 3 * D)
    ).astype(NP_BF16)
    bands_bf = bands.astype(NP_BF16)
    vT_bf = np.ascontiguousarray(v.transpose(0, 2, 1)).astype(NP_BF16)  # [B, D, L]

    # ---- phase 2: folded projection + tap aggregation (device) ----
    if "p2" not in _NC_CACHE:
        _NC_CACHE["p2"] = _build_phase2()
    in_maps = [
        {
            "vT": vT_bf[b],
            "bands": np.ascontiguousarray(bands_bf[b]),
            "w2a": w2a_bf,
            "w2b": w2b_bf,
            "bias": bias_rep,
        }
        for b in range(B)
    ]
    res2 = _run(_NC_CACHE["p2"], in_maps, "phase2")
    return np.stack([res2[b]["out"] for b in range(B)]).astype(np.float32)


# revision 8
# speedup vs baseline: 1.2395x; 1.2395x over previous
"""AutoCorrelation kernel for Trainium2 (8 NeuronCores, SPMD data-parallel over batch).

Math (derived from the reference nn.Module):
  - R = irfft(rfft(Q) * conj(rfft(K))) is a circular cross-correlation; the
    reference reduces it with mean over (heads, ALL lags).  Sum over all lags
    of a circular cross-correlation factorizes:  sum_tau R[tau] =
    (sum_t Q[t]) * (sum_s K[s]).  So the FFT is algebraically unnecessary --
    only column sums of Q and K are needed, and those are linear in the
    column sums of q and k (sum_t(q @ Wq + bq) = (sum_t q) @ Wq + L*bq).
  - The top-k "delays" are channel indices in [0, 64).  The delay aggregation
    sum_i w_i * roll(V, -d_i) commutes with the output projection AND with the
    value projection, so:  out[t] = sum_d coef_d * U[(t+d) % L]  where
    U = v @ (Wv @ Wo), plus bias (bv @ Wo + bo).  The tap sum is a 64-band
    Toeplitz matmul on the tensor engine.
  - The band coefficients sum to exactly 1 (softmax), so the bias can be
    folded into U (out = bands^T (U + bias) = bands^T U + bias) -- the bias
    add rides the PSUM->SBUF cast instead of the conv output path.

Device work:
  phase 1: column sums of q[b], k[b] per core via ones-vector matmuls
           (memory bound; bf16 inputs, fp32 PSUM accumulation)
  phase 2: U = v @ W2 + bias per 128-row tile, then out_i = band1^T U_i +
           band2^T U_{i+1} (circular), stored as bf16 (host upcasts).
           PE warm-up matmuls at kernel start remove the HAM cold-clock
           penalty while the first vT slices are still in flight.
Host work: [8,512]@[512,512] glue matmuls, top-41 of 64, softmax, band build.

Scheduling notes (measured): concurrent DMAs on one HWDGE ring fair-share
bandwidth, so completion tracks issue order only loosely -- keep the DMA
count low and put the first matmuls' operands first; dep-chaining DMA issues
(add_dep_helper) blocks the issuing sequencer and is a net loss.
"""

import sys

sys.path.insert(0, "/opt/trn_rl_repo")

import numpy as np

import concourse.bass as bass
import concourse.bacc as bacc
import concourse.mybir as mybir
import concourse.tile as tile
from concourse.bass_utils import run_bass_kernel_spmd

B, L, D, H = 8, 4096, 512, 8
DK = D // H          # 64
K_TOP = 41           # min(int(5*log(4096)), 64)
NCORES = 8
F32 = mybir.dt.float32
BF16 = mybir.dt.bfloat16
NP_BF16 = mybir.dt.np(BF16)

# set by test.py to collect HW profiles
PROFILE = False
TRACE_DIR = None
LAST_HW_TIME_NS = {"phase1": None, "phase2": None}

_NC_CACHE = {}


def _make_nc():
    return bacc.Bacc(
        "TRN2", target_bir_lowering=False, debug=False, num_devices=NCORES
    )


def _build_phase1():
    """Per-core: sums[0, :512] = sum_t q[t, :], sums[0, 512:] = sum_t k[t, :].

    q/k arrive as bf16; sums accumulate in fp32 PSUM via ones-vector matmuls.
    DMA layout: partition p reads rows 8p..8p+7 of its row-group -- an 8 KB
    contiguous chunk per partition (column sums are row-order invariant).
    """
    nc = _make_nc()
    q = nc.dram_tensor("q", [L, D], BF16, kind="ExternalInput")
    k = nc.dram_tensor("k", [L, D], BF16, kind="ExternalInput")
    sums = nc.dram_tensor("sums", [1, 2 * D], F32, kind="ExternalOutput")

    NSUB = 4                  # 0.5 MB tiles: matmuls track DMA arrival closely
    NBIG = L // (128 * NSUB)  # 8

    with tile.TileContext(nc) as tc:
        with (
            tc.tile_pool(name="singles", bufs=1) as singles,
            tc.tile_pool(name="qk", bufs=4) as qk_pool,
            tc.tile_pool(name="ps", bufs=2, space=bass.MemorySpace.PSUM) as ps_pool,
        ):
            ones = singles.tile([128, 1], BF16)
            nc.any.memset(ones[:], 1.0)

            q_re = q.ap().rearrange("(g p n) d -> g p n d", p=128, n=NSUB)
            k_re = k.ap().rearrange("(g p n) d -> g p n d", p=128, n=NSUB)

            psq = ps_pool.tile([1, D], F32)
            psk = ps_pool.tile([1, D], F32)
            for g in range(NBIG):
                tq = qk_pool.tile([128, NSUB, D], BF16, tag="ldq")
                nc.sync.dma_start(tq[:], q_re[g])
                tk = qk_pool.tile([128, NSUB, D], BF16, tag="ldk")
                nc.scalar.dma_start(tk[:], k_re[g])
                for c in range(NSUB):
                    nc.tensor.matmul(
                        psq[:1, :],
                        ones[:],
                        tq[:, c, :],
                        start=(g == 0 and c == 0),
                        stop=(g == NBIG - 1 and c == NSUB - 1),
                    )
                for c in range(NSUB):
                    nc.tensor.matmul(
                        psk[:1, :],
                        ones[:],
                        tk[:, c, :],
                        start=(g == 0 and c == 0),
                        stop=(g == NBIG - 1 and c == NSUB - 1),
                    )
                if g == NBIG - 1:
                    # q's accumulation is final here; drain it while the
                    # last k matmuls still run so the store overlaps compute
                    oq = singles.tile([1, D], F32, name="oq")
                    nc.vector.tensor_copy(oq[:1, :], psq[:1, :])
                    nc.sync.dma_start(sums.ap()[0:1, 0:D], oq[:1, :])

            ok = singles.tile([1, D], F32, name="ok")
            nc.vector.tensor_copy(ok[:1, :], psk[:1, :])
            nc.sync.dma_start(sums.ap()[0:1, D : 2 * D], ok[:1, :])

    nc.compile()
    return nc


def _build_phase2():
    """Per-core: out[128i + t, n] = sum_s band1[s, t] * U_i[s, n]
                                  + sum_s band2[s, t] * U_{i+1 mod 32}[s, n]
    with U_i = v[128i : 128(i+1), :] @ W2 + bias, from host-transposed vT.
    Output stored bf16 (host upcasts); bias folded into U (bands sum to 1).
    """
    nc = _make_nc()
    vT = nc.dram_tensor("vT", [D, L], BF16, kind="ExternalInput")
    bandsd = nc.dram_tensor("bands", [2, 128, 128], BF16, kind="ExternalInput")
    # host-swizzled: w2[p, cg*D + n] = (Wv@Wo)[cg*128 + p, n] (contiguous rows)
    w2d = nc.dram_tensor("w2", [128, 4 * D], BF16, kind="ExternalInput")
    biasd = nc.dram_tensor("bias", [128, D], F32, kind="ExternalInput")
    out = nc.dram_tensor("out", [L, D], BF16, kind="ExternalOutput")
    warm = nc.dram_tensor("warm", [1, 4], F32, kind="ExternalOutput")

    NBLK = L // 128          # 32 tiles / output blocks
    OSUB = 2                 # output blocks per store DMA
    NCH = 4                  # vT column chunks per channel group
    CHW = L // NCH           # 1024 time steps per chunk
    NPRO = 4                 # cg-major prologue tiles
    PRE_U = 16               # U tiles emitted before the first conv block
    NWARM = 40               # PE warm-up matmuls (HAM ramp) before real work

    with tile.TileContext(nc) as tc:
        with (
            tc.tile_pool(name="singles", bufs=1) as singles,
            tc.tile_pool(name="usb", bufs=PRE_U + 3) as u_pool,
            tc.tile_pool(name="op", bufs=3) as opool,
            tc.tile_pool(name="wps", bufs=1, space=bass.MemorySpace.PSUM) as wps_pool,
            tc.tile_pool(name="ups", bufs=4, space=bass.MemorySpace.PSUM) as ups_pool,
            tc.tile_pool(name="ops", bufs=3, space=bass.MemorySpace.PSUM) as ops_pool,
        ):
            # ---- PE warm-up: no data deps, fills the HAM ramp while the
            # first vT slices are in flight.  Small N so the queue drains
            # just as the first real operands land.  A tiny store keeps it
            # live against DCE. ----
            wtile = singles.tile([128, 128], BF16)
            nc.any.memset(wtile[:], 0.0)
            wps = wps_pool.tile([128, 128], F32)
            for j in range(NWARM):
                nc.tensor.matmul(
                    wps[:],
                    wtile[:],
                    wtile[:],
                    start=(j == 0),
                    stop=(j == NWARM - 1),
                )
            wsb = singles.tile([1, 4], F32)
            nc.vector.tensor_copy(wsb[:], wps[0:1, 0:4])
            nc.sync.dma_start(warm.ap(), wsb[:])

            # ---- DMA schedule (proven layout): one small head chunk per
            # channel group so the first U matmuls start as soon as it lands,
            # then one big chunk for the rest; w2 whole and first on its
            # ring so the prologue can start. ----
            vt_re = vT.ap().rearrange("(c p) t -> c p t", p=128)
            vts = [
                (
                    singles.tile([128, CHW], BF16, name=f"vth{cg}"),
                    singles.tile([128, L - CHW], BF16, name=f"vtr{cg}"),
                )
                for cg in range(4)
            ]
            nc.sync.dma_start(vts[0][0][:], vt_re[0][:, 0:CHW])
            w2_sb = singles.tile([128, 4, D], BF16)
            nc.scalar.dma_start(w2_sb[:], w2d.ap().rearrange("p (c n) -> p c n", c=4))
            nc.sync.dma_start(vts[1][0][:], vt_re[1][:, 0:CHW])
            nc.scalar.dma_start(vts[2][0][:], vt_re[2][:, 0:CHW])
            nc.sync.dma_start(vts[3][0][:], vt_re[3][:, 0:CHW])
            band_sb = singles.tile([128, 2, 128], BF16)
            nc.scalar.dma_start(band_sb[:], bandsd.ap().rearrange("b p t -> p b t"))
            bias_sb = singles.tile([128, D], F32)
            nc.scalar.dma_start(bias_sb[:], biasd.ap())
            for cg in range(4):
                ring = nc.sync if cg % 2 == 0 else nc.scalar
                ring.dma_start(vts[cg][1][:], vt_re[cg][:, CHW:L])

            out_re = out.ap().rearrange("(g n p) d -> g p n d", p=128, n=OSUB)

            TPC = CHW // 128  # tiles in the small head chunk

            def u_mm(ups, i, cg):
                if i < TPC:
                    src = vts[cg][0][:, i * 128 : (i + 1) * 128]
                else:
                    r = i - TPC
                    src = vts[cg][1][:, r * 128 : (r + 1) * 128]
                nc.tensor.matmul(
                    ups[:],
                    src,
                    w2_sb[:, cg, :],
                    start=(cg == 0),
                    stop=(cg == 3),
                )

            def u_cast(ups, i):
                # PSUM -> SBUF with the bias folded in and a bf16 downcast
                usb = u_pool.tile([128, D], BF16, tag="usb", name=f"usb{i}")
                nc.vector.tensor_add(usb[:], ups[:], bias_sb[:])
                return usb

            def u_tile(i):
                ups = ups_pool.tile([128, D], F32, tag="ups", name=f"ups{i}")
                for cg in range(4):
                    u_mm(ups, i, cg)
                return u_cast(ups, i)

            # Prologue: first NPRO tiles in cg-major order so the PE starts as
            # soon as vT head 0 has landed instead of waiting for all heads.
            U = {}
            pro_ups = [
                ups_pool.tile([128, D], F32, tag="ups", name=f"ups{i}")
                for i in range(NPRO)
            ]
            for cg in range(4):
                for i in range(NPRO):
                    u_mm(pro_ups[i], i, cg)
            for i in range(NPRO):
                U[i] = u_cast(pro_ups[i], i)
            u_first = singles.tile([128, D], BF16)
            nc.vector.tensor_copy(u_first[:], U[0][:])

            # ---- more U tiles before the conv stream starts ----
            for i in range(NPRO, PRE_U):
                U[i] = u_tile(i)

            # ---- conv blocks + PSUM->SBUF bf16 copy + stores ----
            ot_tiles = {}
            for i in range(NBLK):
                g, n4 = divmod(i, OSUB)
                if g not in ot_tiles:
                    ot_tiles[g] = opool.tile(
                        [128, OSUB, D], BF16, tag="out", name=f"ot{g}"
                    )
                if i + PRE_U < NBLK:
                    U[i + PRE_U] = u_tile(i + PRE_U)
                u_n = U[i + 1] if i < NBLK - 1 else u_first
                ops = ops_pool.tile([128, D], F32, tag="ops", name=f"ops{i}")
                nc.tensor.matmul(
                    ops[:], band_sb[:, 0, :], U[i][:], start=True, stop=False
                )
                nc.tensor.matmul(
                    ops[:], band_sb[:, 1, :], u_n[:], start=False, stop=True
                )
                del U[i]
                ot = ot_tiles[g]
                nc.scalar.copy(ot[:, n4, :], ops[:])  # ACT: fp32 PSUM -> bf16 SBUF
                if n4 == OSUB - 1:
                    nc.sync.dma_start(out_re[g], ot[:])
                    del ot_tiles[g]

    nc.compile()
    return nc


_RUN_COUNTER = [0]


def _run(nc, in_maps, phase):
    kwargs = {}
    if PROFILE:
        kwargs["trace"] = True
        if TRACE_DIR is not None:
            import os

            _RUN_COUNTER[0] += 1
            d = os.path.join(TRACE_DIR, f"{phase}_{_RUN_COUNTER[0]}")
            os.makedirs(d, exist_ok=True)
            kwargs["tmpdir"] = d
    res = run_bass_kernel_spmd(nc, in_maps, core_ids=list(range(NCORES)), **kwargs)
    LAST_HW_TIME_NS[phase] = res.exec_time_ns
    return res.results


def kernel(q, k, v, Wq, bq, Wk, bk, Wv, bv, Wo, bo):
    q = np.asarray(q, dtype=np.float32)
    k = np.asarray(k, dtype=np.float32)
    v = np.asarray(v, dtype=np.float32)
    Wq, bq, Wk, bk, Wv, bv, Wo, bo = (
        np.asarray(x, dtype=np.float64) for x in (Wq, bq, Wk, bk, Wv, bv, Wo, bo)
    )

    # ---- phase 1: per-batch column sums of q and k (device) ----
    if "p1" not in _NC_CACHE:
        _NC_CACHE["p1"] = _build_phase1()
    q_bf = q.astype(NP_BF16)
    k_bf = k.astype(NP_BF16)
    in_maps = [{"q": q_bf[b], "k": k_bf[b]} for b in range(B)]
    res1 = _run(_NC_CACHE["p1"], in_maps, "phase1")
    sq = np.stack([res1[b]["sums"][0, :D] for b in range(B)]).astype(np.float64)
    sk = np.stack([res1[b]["sums"][0, D:] for b in range(B)]).astype(np.float64)

    # ---- host glue: top-k channel selection + softmax weights ----
    SQ = sq @ Wq + L * bq                       # [B, D]
    SK = sk @ Wk + L * bk
    m = (SQ.reshape(B, H, DK) * SK.reshape(B, H, DK)).sum(axis=1) / (H * L)  # [B, DK]
    mbar = m.mean(axis=0)
    idx = np.argsort(-mbar, kind="stable")[:K_TOP]
    msel = m[:, idx]
    e = np.exp(msel - msel.max(axis=1, keepdims=True))
    w = e / e.sum(axis=1, keepdims=True)        # [B, K_TOP]
    coef = np.zeros((B, DK))
    coef[:, idx] = w

    # Toeplitz bands: out[t] = sum_d coef[d] * U[(t + d) % L]
    s = np.arange(128)[:, None]
    t = np.arange(128)[None, :]
    d1 = s - t
    d2 = s + 128 - t
    m1 = (d1 >= 0) & (d1 < DK)
    m2 = (d2 >= 0) & (d2 < DK)
    bands = np.zeros((B, 2, 128, 128), dtype=np.float64)
    for b in range(B):
        bands[b, 0] = np.where(m1, coef[b][np.clip(d1, 0, DK - 1)], 0.0)
        bands[b, 1] = np.where(m2, coef[b][np.clip(d2, 0, DK - 1)], 0.0)

    W2 = (Wv @ Wo).astype(np.float32)
    bias2 = (bv @ Wo + bo).astype(np.float32)
    bias_rep = np.ascontiguousarray(np.broadcast_to(bias2, (128, D)))
    # swizzle so W2 rows for channel chunk cg sit contiguously per partition
    w2_bf = np.ascontiguousarray(
        W2.reshape(4, 128, D).transpose(1, 0, 2).reshape(128, 4 * D)
    ).astype(NP_BF16)
    bands_bf = bands.astype(NP_BF16)
    vT_bf = np.ascontiguousarray(v.transpose(0, 2, 1)).astype(NP_BF16)  # [B, D, L]

    # ---- phase 2: folded projection + tap aggregation (device) ----
    if "p2" not in _NC_CACHE:
        _NC_CACHE["p2"] = _build_phase2()
    in_maps = [
        {
            "vT": vT_bf[b],
            "bands": np.ascontiguousarray(bands_bf[b]),
            "w2": w2_bf,
            "bias": bias_rep,
        }
        for b in range(B)
    ]
    res2 = _run(_NC_CACHE["p2"], in_maps, "phase2")
    return np.stack([res2[b]["out"] for b in range(B)]).astype(np.float32)
